# revision 9
# baseline (speedup 1.0000x reference)
"""Trainium2 Bass kernel for nn_AttentionModule_50002009260608 — fp8 redesign v2.

B=16, C=512, H=W=24 (HW=576), TF=512, NH=8, CPH=64. Data-parallel: 2 batch/core.

All heavy matmuls in fp8e4m3 with DoubleRow perf mode. Weights scaled x32
host-side; scale ledger (powers of 2, exact):
  Q8,K8,vl8 = 2^5 * true; VT = 2^4 * V; es = C8*exp(l*SCALE) (via exp-scale
  2^-10); ones col = 0.125 so sums row = sum(es)/8; crossn_s = 2^6 * crossn;
  fin main psum = 2^12 * Wr@out_self, cross col = 2^10 * Wr@crossout.
Host applies 2^-12 / 2^-10 and bias = Wr_b + 2*Wr@(Wv@Wm2@t).

v2: AV computed directly as out[c, n_q] with V stationary (VT1 [k, c] incl a
ones column giving softmax sums in a psum ROW; for odd heads the ones column
leads so V rows land on partitions 64:128). The cross-attention value column
rides as es column 576 (crossTm copied in), so no separate N=1 matmuls.
Normalization: reciprocal of the sums row -> [1,578] rrow, DRAM-roundtrip
partition-broadcast to [64,578], single bulk tensor_tensor per 128-channel
chunk converts bf16 unnormalized AV to fp8 oa. QK emitted per m-tile into
2-bank psum tiles (tag ring bufs=2) with q-halves (289, 287); exp per m-tile.
"""

import numpy as np
from contextlib import ExitStack

import concourse.bacc as bacc
import concourse.bass as bass
import concourse.tile as tile
import concourse.mybir as mybir
from concourse import masks
from concourse.bass_utils import run_bass_kernel_spmd

B, C, HW, TF, NH, CPH = 16, 512, 576, 512, 8, 64
NCORES, BPC = 8, B // 8
SCALE = 1.0 / 8.0
F32, BF16, F8 = mybir.dt.float32, mybir.dt.bfloat16, mybir.dt.float8e4
AF = mybir.ActivationFunctionType
OP = mybir.AluOpType
DR = mybir.MatmulPerfMode.DoubleRow
PD = 128
NCC = 4                                   # 128-channel chunks
MT = [(0, 128), (128, 128), (256, 128), (384, 128), (512, 64)]  # key m tiles
QH = [(0, 289), (289, 287)]               # QK query halves (psum cols 0:289)
AH = [(0, 289), (289, 289)]               # AV moving halves over es (incl 576/577)
NHALF = [(0, 288), (288, 288)]            # conv psum halves
FINH = [(0, 288), (288, 289)]             # final halves (incl cross col 576)
OAW = HW + 1                              # out cols (incl crossout col 576)
ESW = HW + 2                              # es cols: 576 q, cross, pad
USE_QK_DR = True
C8 = 1.0625             # half-ulp pre-compensation: fp8 casts truncate on HW
CB = 1.001953125        # same for bf16 casts
LNC8 = float(np.log(1.0625))
CCROSS = 2.0 ** -5 / CB  # rrow value for the cross column


def _body(ctx: ExitStack, tc, d):
    nc = tc.nc

    wt = ctx.enter_context(tc.tile_pool(name="wt", bufs=1))
    act = ctx.enter_context(tc.tile_pool(name="act", bufs=1))
    esp = ctx.enter_context(tc.tile_pool(name="esp", bufs=1))
    pqk = ctx.enter_context(tc.tile_pool(name="pqk", bufs=1, space="PSUM"))
    pcv = ctx.enter_context(tc.tile_pool(name="pcv", bufs=1, space="PSUM"))
    pav = ctx.enter_context(tc.tile_pool(name="pav", bufs=1, space="PSUM"))

    # ---- weights + identities -------------------------------------------
    W = {}
    for wn in ("Wq8", "Wk8", "Wm18", "Wv8", "Wr8"):
        wtile = wt.tile([PD, NCC, C], F8, name=f"{wn}_t")
        nc.sync.dma_start(wtile[:], d[wn].rearrange("(cc p) o -> p cc o", p=PD))
        W[wn] = wtile
    identb = wt.tile([PD, PD], BF16, name="identb")
    masks.make_identity(nc, identb[:])
    lnc8 = wt.tile([PD, 1], F32, name="lnc8")
    nc.vector.memset(lnc8[:], LNC8)

    st = {}

    def emit_loads(b):
        x8 = act.tile([PD, NCC, HW], F8, name=f"x8_{b}", tag="x8", bufs=2)
        nc.sync.dma_start(x8[:], d["x8"][b].rearrange("(cc p) n -> p cc n", p=PD))
        tm8 = act.tile([PD, NCC, NH], F8, name=f"tm8_{b}", tag="tm8", bufs=2)
        nc.sync.dma_start(tm8[:], d["tm8"][b].rearrange("(cc p) h -> p cc h", p=PD))
        st[b] = {"x8": x8, "tm8": tm8}

    def conv_dr(b, Wn, rhs, dst, tag):
        """dst[128, NCC(ot), HW] (fp8) = fp8(Wn^T @ rhs); DR over cc pairs.

        Per-ot 2-bank psum tiles on the shared "cw" ring (bufs=2) so the
        requant of ot overlaps the matmuls of ot+1."""
        for ot in range(NCC):
            p = pqk.tile([PD, 2, 512], F32, tag="cw", bufs=2,
                         name=f"p_{tag}{b}_{ot}")
            for hi, (n0, nsz) in enumerate(NHALF):
                o = p[:, hi, 0:nsz]
                for cp in range(2):
                    nc.tensor.matmul(
                        o, Wn[:, 2 * cp:2 * cp + 2, ot * PD:(ot + 1) * PD],
                        rhs[:, 2 * cp:2 * cp + 2, n0:n0 + nsz],
                        start=(cp == 0), stop=(cp == 1), perf_mode=DR)
            dv = dst[:, ot, :].rearrange("p (a n) -> p a n", a=2)
            nc.vector.tensor_scalar_mul(dv, p[:, 0:2, 0:288], C8)

    def emit_qk_conv(b, which):
        s = st[b]
        t8 = act.tile([PD, NCC, HW], F8, name=f"{which}8_{b}", tag=f"{which}8",
                      bufs=2)
        conv_dr(b, W[f"W{which}8"], s["x8"], t8, f"c{which}")
        s[f"{which}8"] = t8
        # dram roundtrip -> DoubleRow layout [32*g+lane, hi, k, n]
        if USE_QK_DR:
            scr = d[f"scr_{which}"][b]
            nc.sync.dma_start(scr.rearrange("(cc p) n -> p cc n", p=PD), t8[:])
            tdr = act.tile([PD, 2, 2, HW], F8, name=f"{which}dr_{b}",
                           tag=f"{which}dr", bufs=2)
            srcv = scr.rearrange("(g hi k lane) n -> g lane (hi k) n",
                                 g=4, hi=2, k=2, lane=32)
            for gg in range(4):
                nc.sync.dma_start(
                    tdr[32 * gg:32 * gg + 32].rearrange(
                        "p a b n -> p (a b) n"),
                    srcv[gg])
            s[f"{which}dr"] = tdr

    def emit_vl_v(b):
        s = st[b]
        vl8 = act.tile([PD, NCC, HW], F8, name=f"vl8_{b}", tag="vl8", bufs=2)
        conv_dr(b, W["Wm18"], s["x8"], vl8, "cvl")
        # VT1[k, 8 groups of 65]: even head h -> [V(64), ones]; odd head ->
        # [ones, V(64)] so V rows land on psum partitions 64:128.
        VT1 = act.tile([PD, 5, NH * 66], F8, name=f"VT1_{b}", tag="vt1", bufs=2)
        if b < 2:  # ones+pad cols written once per physical buffer (2 bufs)
            v = VT1[:].rearrange("p mi (h c) -> p mi h c", h=NH)
            nc.gpsimd.memset(v[:, :, :, 64:65], 0.125)
            nc.gpsimd.memset(v[:, :, :, 65:66], 0.0)
        for mi, (m0, msz) in enumerate(MT):
            p = pcv.tile([PD, 2, 512], F32, tag="cv", bufs=1, name=f"p_v{b}_{mi}")
            o = p[0:msz, 0, :]
            for cp in range(2):
                nc.tensor.matmul(o, vl8[:, 2 * cp:2 * cp + 2, m0:m0 + msz],
                                 W["Wv8"][:, 2 * cp:2 * cp + 2, :],
                                 start=(cp == 0), stop=(cp == 1), perf_mode=DR)
            vdst = VT1[0:msz, mi, :].rearrange("p (h c) -> p h c", h=NH)[:, :, 0:64]
            nc.vector.tensor_scalar_mul(
                vdst, p[0:msz, 0, :].rearrange("p (h c) -> p h c", h=NH),
                2.0 ** -6 * C8)
        s["VT1"] = VT1

    def emit_cross(b):
        s = st[b]
        p = pcv.tile([NH, 2, 512], F32, tag="cv", bufs=1, name=f"p_cr{b}")
        for hi, (n0, nsz) in enumerate(NHALF):
            o = p[:, hi, 0:nsz]
            for cc in range(NCC):
                nc.tensor.matmul(o, s["tm8"][:, cc, :],
                                 s["x8"][:, cc, n0:n0 + nsz],
                                 start=(cc == 0), stop=(cc == NCC - 1))
        crosse = act.tile([NH, HW], F32, name=f"crosse{b}", tag="crosse")
        csum = act.tile([NH, 1], F32, name=f"csum{b}", tag="csum")
        nc.scalar.activation(
            crosse[:].rearrange("p (a n) -> p a n", a=2), p[:, :, 0:288],
            AF.Exp, scale=SCALE / 32.0, accum_out=csum[:])
        crec = act.tile([NH, 1], F32, name=f"crec{b}", tag="crec")
        nc.vector.reciprocal(crec[:], csum[:])
        crossn_s = act.tile([NH, HW], BF16, name=f"crossn{b}", tag="crossn")
        nc.vector.tensor_scalar(crossn_s[:], crosse[:], crec[:], 64.0 * CB,
                                op0=OP.mult, op1=OP.mult)
        # transpose [8, m] -> [m, 8] then fp8 copy
        crossTm = act.tile([PD, 5, NH], F8, name=f"crossTm{b}", tag="crossTm")
        for mi, (m0, msz) in enumerate(MT):
            pt = pav.tile([PD, 2, 512], BF16, tag="av", bufs=1,
                          name=f"p_ct{b}_{mi}")
            nc.tensor.transpose(pt[0:msz, 0, 0:NH], crossn_s[:, m0:m0 + msz],
                                identb[0:NH, 0:NH])
            nc.vector.tensor_scalar_mul(crossTm[0:msz, mi, :],
                                        pt[0:msz, 0, 0:NH], C8)
        s["crossTm"] = crossTm
        s["outall"] = act.tile([PD, NCC, ESW], F8, name=f"oa{b}", tag="oa",
                               bufs=2)
        s["oaun"] = act.tile([PD, NCC, ESW], BF16, name=f"oaun{b}", tag="oaun",
                             bufs=2)

    def _qk_ops(b, h):
        s = st[b]
        g, hi = h // 2, h % 2

        def f(m0, msz, n0, nsz):
            if USE_QK_DR:
                kdr, qdr = s["kdr"], s["qdr"]
                return (kdr[32 * g:32 * g + 32, hi, :, m0:m0 + msz],
                        qdr[32 * g:32 * g + 32, hi, :, n0:n0 + nsz], DR,
                        (32 * g, 0))
            p0 = 64 * (h % 2)
            return (s["k8"][p0:p0 + CPH, h // 2, m0:m0 + msz],
                    s["q8"][p0:p0 + CPH, h // 2, n0:n0 + nsz], None, (p0, 0))
        return f

    def emit_qk_mt(b, h, mi):
        """QK for key m-tile mi of head h -> exp into es[:, mi, 0:578]."""
        s = st[b]
        if mi == 0:
            s[f"es{h}"] = esp.tile([PD, 5, ESW], F8, name=f"es{b}_{h}",
                                   tag="es", bufs=2)
        es = s[f"es{h}"]
        qk = _qk_ops(b, h)
        m0, msz = MT[mi]
        qp = pqk.tile([PD, 2, 512], F32, tag="cw", bufs=2,
                      name=f"p_qk{b}_{h}_{mi}")
        for hi2, (n0, nsz) in enumerate(QH):
            lhs, rhs, pm, tp = qk(m0, msz, n0, nsz)
            nc.tensor.matmul(qp[0:msz, hi2, 0:nsz], lhs, rhs,
                             start=True, stop=True, perf_mode=pm,
                             tile_position=tp, skip_group_check=True)
        nc.scalar.activation(
            es[0:msz, mi, :].rearrange("p (a n) -> p a n", a=2),
            qp[0:msz, 0:2, 0:289],
            AF.Exp, scale=SCALE / 1024.0 / (C8 * C8), bias=lnc8[0:msz, :])

    def emit_escross(b, h):
        # cross-attention value column rides as es col 576 (after exps).
        s = st[b]
        es = s[f"es{h}"]
        nc.vector.tensor_scalar_mul(es[:, 0:5, HW:HW + 1],
                                    s["crossTm"][:, :, h:h + 1], 1.0)

    def emit_av(b, h, ci):
        """AV chunk ci for head h: out[c(+sums row), q] with V stationary."""
        s = st[b]
        es = s[f"es{h}"]
        if ci == 0:
            s[f"av{h}"] = pav.tile([PD, 2, 512], F32, tag="av", bufs=1,
                                   name=f"p_av{b}_{h}")
        av = s[f"av{h}"]
        vsl = slice(h * 66, (h + 1) * 66)
        for hi, (q0, qn) in enumerate(AH):
            o = av[0:66, hi, 0:qn]
            if ci < 2:
                mi = 2 * ci
                nc.tensor.matmul(
                    o, s["VT1"][:, mi:mi + 2, vsl], es[:, mi:mi + 2, q0:q0 + qn],
                    start=(ci == 0), stop=False, perf_mode=DR,
                    skip_group_check=True)
            else:
                nc.tensor.matmul(
                    o, s["VT1"][0:64, 4, vsl], es[0:64, 4, q0:q0 + qn],
                    start=False, stop=True, skip_group_check=True)

    def emit_drain(b, h):
        """av psum -> bf16 unnormalized oa + reciprocal row -> dram."""
        s = st[b]
        av = s[f"av{h}"]
        v0 = 64 * (h % 2)        # V rows base in the oa chunk
        nc.vector.tensor_scalar_mul(
            s["oaun"][v0:v0 + 64, h // 2, 0:ESW].rearrange(
                "p (a n) -> p a n", a=2),
            av[0:64, 0:2, 0:289], C8 * CB)
        rrow = act.tile([1, ESW], F32, name=f"rr{b}_{h}", tag="rrow", bufs=2)
        nc.vector.reciprocal(rrow[:].rearrange("p (a n) -> p a n", a=2),
                             av[64:65, 0:2, 0:289])
        nc.vector.memset(rrow[0:1, HW:HW + 1], CCROSS)
        par = h % 2
        nc.sync.dma_start(d["rscr"][b, h // 2, par:par + 1], rrow[0:1, :])

    def emit_bulk(b, ot):
        """broadcast the two rrows of chunk ot and produce fp8 oa chunk."""
        s = st[b]
        rb = act.tile([PD, ESW], F32, name=f"rb{b}_{ot}", tag="rb", bufs=2)
        nc.sync.dma_start(rb[0:64, :],
                          d["rscr"][b, ot, 0:1].partition_broadcast(64))
        nc.sync.dma_start(rb[64:128, :],
                          d["rscr"][b, ot, 1:2].partition_broadcast(64))
        nc.vector.tensor_tensor(s["outall"][:, ot, :], s["oaun"][:, ot, :],
                                rb[:], OP.mult)

    def emit_fin(b, ots):
        s = st[b]
        oa = s["outall"]
        if "fin" not in s:
            s["fin"] = act.tile([PD, NCC, OAW], BF16, name=f"fin{b}", tag="fin",
                                bufs=2)
        fin = s["fin"]
        for ot in ots:
            p = pcv.tile([PD, 2, 512], F32, tag="cv", bufs=1,
                         name=f"p_f{b}_{ot}")
            for hi, (n0, nsz) in enumerate(FINH):
                o = p[:, hi, 0:nsz]
                for cp in range(2):
                    nc.tensor.matmul(
                        o, W["Wr8"][:, 2 * cp:2 * cp + 2, ot * PD:(ot + 1) * PD],
                        oa[:, 2 * cp:2 * cp + 2, n0:n0 + nsz],
                        start=(cp == 0), stop=(cp == 1), perf_mode=DR)
            nc.vector.tensor_scalar_mul(fin[:, ot, 0:288], p[:, 0, 0:288], CB)
            nc.vector.tensor_scalar_mul(fin[:, ot, 288:OAW], p[:, 1, 0:289], CB)
            nc.sync.dma_start(
                d["out"][b, ot * PD:(ot + 1) * PD, :], fin[:, ot, :])

    # ---- schedule: AV chunks of head h-1 between head h's QK m-tiles ----
    def heads(b, fin_b=None):
        for h in range(NH):
            emit_qk_mt(b, h, 0)
            emit_qk_mt(b, h, 1)
            if h > 0:
                emit_av(b, h - 1, 0)
            emit_qk_mt(b, h, 2)
            if h > 0:
                emit_av(b, h - 1, 1)
            emit_qk_mt(b, h, 3)
            if h > 0:
                emit_av(b, h - 1, 2)
                emit_drain(b, h - 1)
            emit_qk_mt(b, h, 4)
            emit_escross(b, h)
            if h > 0 and h % 2 == 0:
                emit_bulk(b, h // 2 - 1)
                if fin_b is not None:
                    emit_fin(fin_b, [h // 2 - 1])
        for ci in range(3):
            emit_av(b, NH - 1, ci)
        emit_drain(b, NH - 1)
        emit_bulk(b, 3)
        if fin_b is not None:
            emit_fin(fin_b, [3])

    emit_loads(0)
    emit_qk_conv(0, "q")
    emit_qk_conv(0, "k")
    emit_loads(1)
    emit_vl_v(0)
    emit_cross(0)
    heads(0)
    emit_qk_conv(1, "q")
    emit_qk_conv(1, "k")
    emit_vl_v(1)
    emit_cross(1)
    heads(1, fin_b=0)
    emit_fin(1, range(NCC))


_CACHE = {}


def _build():
    if "nc" in _CACHE:
        return _CACHE["nc"], _CACHE["out"]
    nc = bacc.Bacc("TRN2", target_bir_lowering=False, debug=False,
                   num_devices=NCORES)
    d = {
        "x8": nc.dram_tensor("x8", [BPC, C, HW], F8, kind="ExternalInput").ap(),
        "tm8": nc.dram_tensor("tm8", [BPC, C, NH], F8,
                              kind="ExternalInput").ap(),
        "out": nc.dram_tensor("out", [BPC, C, OAW], BF16,
                              kind="ExternalOutput").ap(),
        "scr_q": nc.dram_tensor("scr_q", [BPC, C, HW], F8, kind="Internal").ap(),
        "scr_k": nc.dram_tensor("scr_k", [BPC, C, HW], F8, kind="Internal").ap(),
        "rscr": nc.dram_tensor("rscr", [BPC, NCC, 2, ESW], F32,
                               kind="Internal").ap(),
    }
    for wn in ("Wq8", "Wk8", "Wm18", "Wv8", "Wr8"):
        d[wn] = nc.dram_tensor(wn, [C, C], F8, kind="ExternalInput").ap()
    with tile.TileContext(nc) as tc:
        with ExitStack() as ctx:
            _body(ctx, tc, d)
    nc.compile()
    _CACHE["nc"], _CACHE["out"] = nc, d["out"].tensor.name
    return nc, _CACHE["out"]


def _prep_inputs(x, t, Wk, Wq, Wt_w, Wt_b, Wm, Wv, Wr_w, Wr_b):
    f = np.float32
    f8 = mybir.dt.np(F8)
    x = np.asarray(x, f).reshape(B, C, HW)
    t = np.asarray(t, f)
    Wm1 = np.asarray(Wm, f)[:, :C]
    t_m = t @ np.asarray(Wt_w, f).T + np.asarray(Wt_b, f)
    tm_blk = np.zeros((B, C, NH), f)
    for h in range(NH):
        tm_blk[:, h * CPH:(h + 1) * CPH, h] = t_m[:, h * CPH:(h + 1) * CPH] * 32.0
    vb = (t @ np.asarray(Wm, f)[:, C:].T) @ np.asarray(Wv, f).T
    bias_host = (np.asarray(Wr_b, f)[None, :]
                 + 2.0 * (vb @ np.asarray(Wr_w, f).T))          # [B, C]
    com = {
        "Wq8": np.ascontiguousarray(np.asarray(Wq, f).T * 32).astype(f8),
        "Wk8": np.ascontiguousarray(np.asarray(Wk, f).T * 32).astype(f8),
        "Wm18": np.ascontiguousarray(Wm1.T * 32).astype(f8),
        "Wv8": np.ascontiguousarray(np.asarray(Wv, f).T * 32).astype(f8),
        "Wr8": np.ascontiguousarray(np.asarray(Wr_w, f).T * 32).astype(f8),
    }
    maps = []
    for c in range(NCORES):
        sl = slice(c * BPC, (c + 1) * BPC)
        m = dict(com)
        m["x8"] = np.ascontiguousarray(x[sl]).astype(f8)
        m["tm8"] = np.ascontiguousarray(tm_blk[sl]).astype(f8)
        maps.append(m)
    return maps, bias_host


def kernel(x, t, Wk, Wq, Wt_w, Wt_b, Wm, Wv, Wr_w, Wr_b, _trace=False):
    nc, out_name = _build()
    maps, bias_host = _prep_inputs(x, t, Wk, Wq, Wt_w, Wt_b, Wm, Wv, Wr_w, Wr_b)
    res = run_bass_kernel_spmd(nc, maps, core_ids=list(range(NCORES)),
                               trace=_trace)
    raw = np.concatenate([res.results[c][out_name].astype(np.float32)
                          for c in range(NCORES)], axis=0)   # [B, C, 577]
    c8, cb = 1.0625, 1.001953125
    dmain = 2.0 ** 12 * c8 ** 3 * cb ** 2
    dcross = 2.0 ** 10 * c8 ** 4 * cb ** 2
    out = (raw[:, :, :HW] / dmain
           + raw[:, :, HW:HW + 1] / dcross
           + bias_host[:, :, None]).astype(np.float32)
    if _trace:
        kernel.last_results = res
    return out.reshape(B, C, 24, 24)
